# revision 4
# baseline (speedup 1.0000x reference)
"""Trainium2 Bass kernel for multi-head attention with RoPE (nn_Attention).

Reference computation (B=1, N=2048, D=1024, 16 heads, hd=64):
    q = x @ wq.T; k = x @ wk.T; v = x @ wv.T      (reshaped to heads)
    q, k = rope(q), rope(k)
    out = softmax(q k^T / sqrt(hd)) v              (non-causal, full)
    return (out reshaped) @ wp.T

Sharding: tensor-parallel over heads — each of the 8 cores owns 2 heads for
QKV projection + SDPA, then an AllToAll redistributes the attention output
so each core computes the final projection for its 256 sequence rows with
the full wp.

All matmul operands are bf16 (converted host-side, so DMA traffic is half
of f32 and no on-chip rounding copies are needed); accumulation stays f32
in PSUM. The softmax exp runs on the scalar engine from f32 PSUM logits.

Self-contained: only imports numpy + the concourse stack available in the
execution environment. kernel(**inputs) takes the full unsharded inputs and
returns the full output.
"""
import numpy as np

DIM = 1024
NHEADS = 16
HD = 64
SEQ = 2048
NCORES = 8
ROPE_BASE = 10000.0
HPC = NHEADS // NCORES      # heads per core = 2
CH = HPC * HD               # channels per core = 128
QCH = 512                   # q-chunk (free dim of S/P tiles)
NQC = SEQ // QCH            # 4
NKT = SEQ // 128            # 16 k-tiles
DCH = DIM // 128            # 8 contraction chunks

_CACHE = {}
_PARTS_MODE = "ab"


def _np_bf16():
    import concourse.mybir as mybir
    return mybir.dt.np(mybir.dt.bfloat16)


def _rope_tables():
    inv = 1.0 / (ROPE_BASE ** (np.arange(0, HD, 2, dtype=np.float64) / HD))
    t = np.arange(SEQ, dtype=np.float64)
    freqs = np.outer(t, inv)                      # [SEQ, 32]
    emb = np.concatenate([freqs, freqs], 1)       # [SEQ, 64]
    cosT = np.cos(emb).T                          # [64, SEQ]
    sinT = np.sin(emb).T
    sig = (np.arange(HD) + 32) % HD
    sT = sinT[sig]                                # shifted sin
    cos2 = np.concatenate([cosT, cosT], 0)        # [128, SEQ] (2 heads)
    s2 = np.concatenate([sT, sT], 0)
    return cos2, s2


def _r2t():
    # rotate-half matrix R (per head), block-diagonal over the 2 heads; we
    # pass R2.T as the stationary matmul operand.
    R = np.zeros((HD, HD), np.float64)
    for j in range(32):
        R[j, j + 32] = -1.0
        R[j + 32, j] = 1.0
    R2 = np.zeros((CH, CH), np.float64)
    R2[0:HD, 0:HD] = R
    R2[HD:CH, HD:CH] = R
    return np.ascontiguousarray(R2.T)


def _build(nrep=1, n_cores=NCORES, with_c=True, parts="ab"):
    global _PARTS_MODE
    _PARTS_MODE = parts
    import concourse.mybir as mybir
    import concourse.tile as tile
    from concourse import bacc
    from concourse.masks import make_identity

    F32 = mybir.dt.float32
    F32R = mybir.dt.float32r
    BF16 = mybir.dt.bfloat16
    EXP = mybir.ActivationFunctionType.Exp

    nc = bacc.Bacc("TRN2", target_bir_lowering=False, debug=False,
                   num_devices=n_cores)

    xt_ext = nc.dram_tensor("xt", [DIM, SEQ], BF16, kind="ExternalInput")
    wq_ext = nc.dram_tensor("wq_t", [DIM, CH], BF16, kind="ExternalInput")
    wk_ext = nc.dram_tensor("wk_t", [DIM, CH], BF16, kind="ExternalInput")
    wv_ext = nc.dram_tensor("wv_t", [DIM, CH], BF16, kind="ExternalInput")
    wp_ext = nc.dram_tensor("wp_t", [DIM, DIM], BF16, kind="ExternalInput")
    ck_ext = nc.dram_tensor("cos_k", [CH, SEQ], BF16, kind="ExternalInput")
    sk_ext = nc.dram_tensor("sin_k", [CH, SEQ], BF16, kind="ExternalInput")
    r2t_ext = nc.dram_tensor("r2t", [CH, CH], BF16, kind="ExternalInput")
    out_ext = nc.dram_tensor("out", [SEQ // NCORES, DIM], F32,
                             kind="ExternalOutput")
    a2a_in = nc.dram_tensor("a2a_in", [NCORES, CH, SEQ // NCORES], BF16)
    a2a_out = nc.dram_tensor("a2a_out", [NCORES, CH, SEQ // NCORES], BF16)

    with tile.TileContext(nc) as tc:

        def stage_ab(Qp, Kp, Vsb, onescol, parts="ab"):
            # One unified scope for projections + attention so the Tile
            # scheduler can overlap attention chunks with later Q chunks.
            # PSUM budget (8 banks): big (2-bank slots x2) + small (1-bank
            # x2) + oaug (1-bank x2).
            with (
                tc.tile_pool(name="stA", bufs=1) as A_sb,
                tc.tile_pool(name="stA2", bufs=2) as A_db,
                tc.tile_pool(name="psBig", bufs=2, space="PSUM") as psBig,
                tc.tile_pool(name="psSm", bufs=2, space="PSUM") as psSm,
                tc.tile_pool(name="psO", bufs=2, space="PSUM") as psO,
                tc.tile_pool(name="stB", bufs=3) as B_db,
                tc.tile_pool(name="stBs", bufs=3) as B_sm,
            ):
                if parts in ("b", "s"):
                    _attention(Qp, Kp, Vsb, onescol, A_db, B_db, B_sm,
                               psBig, psSm, psO, None, None, None, False)
                    return
                aux1 = A_sb.tile([128, HD], BF16, tag="aux1")
                nc.vector.memset(aux1[:], 1.0)
                nc.vector.tensor_copy(onescol[:], aux1[:])
                nc.vector.tensor_copy(
                    Vsb[:, :, :, HD],
                    aux1[:, 0:NKT * HPC].rearrange("p (k h) -> p k h", h=HPC))

                # ---- stage A inputs: everything arrives bf16 from DRAM,
                # no rounding copies needed.
                wq = A_sb.tile([128, DCH, CH], BF16, tag="wq")
                wk = A_sb.tile([128, DCH, CH], BF16, tag="wk")
                wv = A_sb.tile([128, DCH, CH], BF16, tag="wv")
                xt = A_sb.tile([128, DCH, SEQ], BF16, tag="xt")
                xt_r = xt_ext.rearrange("(c p) n -> p c n", p=128)
                r2t = A_sb.tile([CH, CH], BF16, tag="r2t")
                ck = A_sb.tile([CH, SEQ], BF16, tag="ck")
                sk = A_sb.tile([CH, SEQ], BF16, tag="sk")
                nc.sync.dma_start(
                    out=wk[:], in_=wk_ext.rearrange("(c p) j -> p c j", p=128))
                nc.sync.dma_start(out=r2t[:], in_=r2t_ext[:])
                nc.sync.dma_start(
                    out=wv[:], in_=wv_ext.rearrange("(c p) j -> p c j", p=128))
                for d in range(DCH):
                    nc.sync.dma_start(out=xt[:, d, :], in_=xt_r[:, d, :])
                nc.sync.dma_start(out=sk[:], in_=sk_ext[:])
                nc.sync.dma_start(out=ck[:], in_=ck_ext[:])
                nc.sync.dma_start(
                    out=wq[:], in_=wq_ext.rearrange("(c p) j -> p c j", p=128))
                ident = A_sb.tile([128, 128], F32, tag="ident")
                make_identity(nc, ident[:])
                identr = A_sb.tile([128, 128], BF16, tag="identr")
                nc.vector.tensor_copy(identr[:], ident[:])

                # ---- projections: K first, then V, then Q — attention
                # q-chunks only need Q' chunk-by-chunk, so emitting Q last
                # lets attention overlap the tail of the projections.
                def qk_proj(w_sb, cos_sb, sin_sb, dst, qc):
                    sl = slice(qc * QCH, (qc + 1) * QCH)
                    ps_q = psSm.tile([CH, QCH], F32, tag="sm")
                    for d in range(DCH):
                        nc.tensor.matmul(ps_q[:], w_sb[:, d, :],
                                         xt[:, d, sl],
                                         start=(d == 0), stop=(d == DCH - 1))
                    qs = A_db.tile([CH, QCH], BF16, tag="qs")
                    nc.vector.tensor_mul(qs[:], ps_q[:], sin_sb[:, sl])
                    qct = A_db.tile([CH, QCH], BF16, tag="qct")
                    nc.vector.tensor_mul(qct[:], ps_q[:], cos_sb[:, sl])
                    nc.tensor.matmul(ps_q[:], r2t[:], qs[:],
                                     start=True, stop=True)
                    nc.vector.tensor_add(dst[:, sl], qct[:], ps_q[:])

                # K and V projections, d-outer: all 8 chunk-accumulators
                # live at once (4 K halves in the two 2-bank "big" slots,
                # 4 V chunks in the four 1-bank slots), so the first xt
                # d-chunk to arrive immediately feeds 8 matmuls.
                kacc0 = psBig.tile([128, HPC, QCH], F32, tag="big")
                kacc1 = psBig.tile([128, HPC, QCH], F32, tag="big")
                vacc0 = psSm.tile([CH, QCH], F32, tag="sm")
                vacc1 = psSm.tile([CH, QCH], F32, tag="sm")
                vacc2 = psO.tile([CH, QCH], F32, tag="oaug")
                vacc3 = psO.tile([CH, QCH], F32, tag="oaug")
                kaccs = [kacc0[:, 0, :], kacc0[:, 1, :],
                         kacc1[:, 0, :], kacc1[:, 1, :]]
                vaccs = [vacc0, vacc1, vacc2, vacc3]
                for d in range(DCH):
                    st, sp = d == 0, d == DCH - 1
                    for c in range(NQC):
                        slc = slice(c * QCH, (c + 1) * QCH)
                        nc.tensor.matmul(kaccs[c], wk[:, d, :], xt[:, d, slc],
                                         start=st, stop=sp)
                        nc.tensor.matmul(vaccs[c][:], wv[:, d, :],
                                         xt[:, d, slc], start=st, stop=sp)

                # RoPE for K: the rot matmuls overwrite the K-accumulator
                # banks in place (start=True) after both DVE reads. The two
                # chunks of each accumulator tile are contiguous, so the DVE
                # muls/adds run at 1024 width (half the ops).
                for pair, kacc in ((0, kacc0), (1, kacc1)):
                    sl2 = slice(pair * 2 * QCH, (pair + 1) * 2 * QCH)
                    kview = kacc[:].rearrange("p a b -> p (a b)")
                    qs = A_db.tile([CH, 2 * QCH], BF16, tag="qs")
                    nc.vector.tensor_mul(qs[:], kview, sk[:, sl2])
                    qct = A_db.tile([CH, 2 * QCH], BF16, tag="qct")
                    nc.vector.tensor_mul(qct[:], kview, ck[:, sl2])
                    for half in range(2):
                        nc.tensor.matmul(
                            kacc[:, half, :], r2t[:],
                            qs[:, half * QCH:(half + 1) * QCH],
                            start=True, stop=True)
                    nc.vector.tensor_add(Kp[:, sl2], qct[:], kview)

                qk_proj(wq, ck, sk, Qp, 0)

                # V: copy out of psum on the DVE (gpsimd has no PSUM port;
                # the scalar engine must stay free for exp), then
                # PE-transpose into Vsb (emitted after Q0 so the
                # attention-critical path starts sooner)
                for c in range(NQC):
                    vt = A_db.tile([CH, QCH], BF16, tag="vt")
                    nc.vector.tensor_copy(vt[:], vaccs[c][:])
                    for b in range(QCH // 128):
                        kti = c * (QCH // 128) + b
                        ps_t = psSm.tile([128, 128], BF16, tag="sm")
                        nc.tensor.transpose(
                            ps_t[:], vt[:, b * 128:(b + 1) * 128], identr[:])
                        nc.vector.tensor_copy(
                            Vsb[:, kti, :, 0:HD],
                            ps_t[:].rearrange("p (h j) -> p h j", h=HPC))

                if parts in ("ab", "b", "s"):
                    _attention(Qp, Kp, Vsb, onescol, A_db, B_db, B_sm,
                               psBig, psSm, psO, qk_proj, wq, (ck, sk), True)
                else:
                    for qc in range(1, NQC):
                        qk_proj(wq, ck, sk, Qp, qc)

        def _attention(Qp, Kp, Vsb, onescol, A_db, B_db, B_sm,
                       psBig, psSm, psO, qk_proj, wq, cs, interleave):
                # ---- attention per head pair, interleaved with the
                # projection of the next Q chunk (hides Q under exp) ----
                s_only = (_PARTS_MODE == "s")

                def emit_tail(o_ps, qc):
                    # softmax normalization + a2a scatter for chunk qc;
                    # deferred into the next chunk's exp shadow so the PE
                    # never stalls on the DVE reciprocal at a boundary.
                    for h in range(HPC):
                        rec = B_sm.tile([HD + 1, QCH], F32R, tag="rec")
                        with nc.allow_low_precision(
                                reason="f32r is fp32-width; rounding only"):
                            nc.vector.reciprocal(rec[HD:HD + 1, :],
                                                 o_ps[h][HD:HD + 1, :])
                        rb_ps = psSm.tile([HD, QCH], F32, tag="sm")
                        nc.tensor.matmul(rb_ps[:], onescol[HD:HD + 1, :],
                                         rec[HD:HD + 1, :],
                                         start=True, stop=True,
                                         tile_position=(HD, 0))
                        rb = B_sm.tile([HD, QCH], F32R, tag="rb_sb")
                        nc.vector.tensor_copy(rb[:], rb_ps[:])
                        on = B_db.tile([HD, QCH], BF16, tag="on")
                        nc.vector.tensor_mul(on[:], o_ps[h][0:HD, :], rb[:])
                        # one strided DMA covers both destination cores
                        nc.sync.dma_start(
                            out=a2a_in[2 * qc:2 * qc + 2,
                                       h * HD:(h + 1) * HD, :]
                            .rearrange("r p n -> p r n"),
                            in_=on[:].rearrange("p (r n) -> p r n", r=2))

                pending = None
                for qc in range(NQC):
                    sl = slice(qc * QCH, (qc + 1) * QCH)
                    # software-pipelined emission: S(kt+1) is emitted
                    # before O(kt) so the in-order PE fills the exp(kt)
                    # shadow with the next S pair instead of stalling.
                    def emit_s(kt):
                        s_ps = psBig.tile([128, HPC, QCH], F32, tag="big")
                        for h in range(HPC):
                            nc.tensor.matmul(
                                s_ps[:, h, :],
                                Kp[h * HD:(h + 1) * HD,
                                   kt * 128:(kt + 1) * 128],
                                Qp[h * HD:(h + 1) * HD, sl],
                                start=True, stop=True,
                                tile_position=(h * HD, 0))
                        p_sb = B_db.tile([128, HPC, QCH], BF16, tag="p")
                        nc.scalar.activation(out=p_sb[:], in_=s_ps[:], func=EXP)
                        return p_sb

                    def emit_o(kt, p_sb):
                        for h in range(HPC):
                            nc.tensor.matmul(
                                o_ps[h][:], Vsb[:, kt, h, :], p_sb[:, h, :],
                                start=(kt == 0), stop=(kt == NKT - 1))

                    p_prev = emit_s(0)
                    if pending is not None:
                        emit_tail(*pending)
                        pending = None
                    if interleave and qc + 1 < NQC:
                        # Q(qc+1) projection rides in the exp shadows of this
                        # chunk's early k-tiles (emitted after S(0) so it
                        # cannot delay the attention-critical path).
                        qk_proj(wq, cs[0], cs[1], Qp, qc + 1)
                    o_ps = None
                    if not s_only:
                        o_ps0 = psO.tile([HD + 1, QCH], F32, tag="oaug")
                        o_ps1 = psO.tile([HD + 1, QCH], F32, tag="oaug")
                        o_ps = [o_ps0, o_ps1]
                    for kt in range(1, NKT):
                        p_cur = emit_s(kt)
                        if not s_only:
                            emit_o(kt - 1, p_prev)
                        p_prev = p_cur
                    if not s_only:
                        emit_o(NKT - 1, p_prev)
                        pending = (o_ps, qc)
                if pending is not None:
                    emit_tail(*pending)

        def stage_c():
            with (
                tc.tile_pool(name="stC", bufs=1) as C_sb,
                tc.tile_pool(name="stC2", bufs=2) as C_db,
                tc.tile_pool(name="psC", bufs=2, space="PSUM") as psC,
            ):
                wp = C_sb.tile([128, DCH, DIM], BF16, tag="wp")
                nc.sync.dma_start(
                    out=wp[:], in_=wp_ext.rearrange("(s p) o -> p s o", p=128))
                nc.gpsimd.collective_compute(
                    "AllToAll", mybir.AluOpType.bypass,
                    replica_groups=[list(range(NCORES))],
                    ins=[a2a_in[:]], outs=[a2a_out[:]])
                ga = C_sb.tile([CH, NCORES, 256], BF16, tag="ga")
                # per-src gather: the first projection matmul starts after
                # one chunk instead of the whole payload (subtile deps gate
                # per region)
                for r in range(NCORES):
                    nc.sync.dma_start(out=ga[:, r, :], in_=a2a_out[r])
                for nt in range(2):
                    for oc in range(2):
                        pp = psC.tile([128, 512], F32, tag="pp")
                        for src in range(NCORES):
                            nc.tensor.matmul(
                                pp[:], ga[:, src, nt * 128:(nt + 1) * 128],
                                wp[:, src, oc * 512:(oc + 1) * 512],
                                start=(src == 0), stop=(src == NCORES - 1))
                        ob = C_db.tile([128, 512], F32, tag="ob")
                        nc.scalar.copy(ob[:], pp[:])
                        nc.sync.dma_start(
                            out=out_ext[nt * 128:(nt + 1) * 128,
                                        oc * 512:(oc + 1) * 512],
                            in_=ob[:])

        with tc.tile_pool(name="persist", bufs=1) as P1:
            Qp = P1.tile([CH, SEQ], BF16, tag="Qp")
            Kp = P1.tile([CH, SEQ], BF16, tag="Kp")
            Vsb = P1.tile([128, NKT, HPC, HD + 1], BF16, tag="Vsb")
            onescol = P1.tile([128, HD], F32R, tag="onescol")
            if nrep == 1:
                if parts in ("b", "s"):
                    stage_ab(Qp, Kp, Vsb, onescol, "a")
                stage_ab(Qp, Kp, Vsb, onescol, parts)
                if with_c:
                    stage_c()
            else:
                # timing build: loop stages A+B (a collective inside a For_i
                # desyncs the mesh), run stage C once after the loop.
                if parts in ("b", "s"):
                    stage_ab(Qp, Kp, Vsb, onescol, "a")
                with tc.For_i(0, nrep, 1) as _i:
                    stage_ab(Qp, Kp, Vsb, onescol, parts)
                if with_c:
                    stage_c()

    nc.compile()
    return nc


def _get_nc(nrep=1, n_cores=NCORES, with_c=True, parts="ab"):
    key = ("nc", nrep, n_cores, with_c, parts)
    if key not in _CACHE:
        _CACHE[key] = _build(nrep, n_cores, with_c, parts)
    return _CACHE[key]


def _prep_in_maps(x, wq, wk, wv, wp):
    bf16 = _np_bf16()
    x2 = np.asarray(x, np.float32).reshape(SEQ, DIM)
    xt = np.ascontiguousarray(x2.T).astype(bf16)
    wq = np.asarray(wq, np.float64)
    wk = np.asarray(wk, np.float64)
    wv = np.asarray(wv, np.float64)
    wp = np.asarray(wp, np.float32)
    cos2, s2 = _rope_tables()
    scale = 1.0 / np.sqrt(HD)
    wq = wq * scale
    ck = np.ascontiguousarray(cos2).astype(bf16)
    sk = np.ascontiguousarray(s2).astype(bf16)
    r2t = _r2t().astype(bf16)
    wpt = np.ascontiguousarray(wp.T).astype(bf16)
    maps = []
    for c in range(NCORES):
        ch = slice(c * CH, (c + 1) * CH)
        maps.append({
            "xt": xt,
            "wq_t": np.ascontiguousarray(wq[ch, :].T).astype(bf16),
            "wk_t": np.ascontiguousarray(wk[ch, :].T).astype(bf16),
            "wv_t": np.ascontiguousarray(wv[ch, :].T).astype(bf16),
            "wp_t": wpt,
            "cos_k": ck, "sin_k": sk,
            "r2t": r2t,
        })
    return maps


def kernel(x, wq, wk, wv, wp):
    from concourse.bass_utils import run_bass_kernel_spmd

    nc = _get_nc(1)
    maps = _prep_in_maps(x, wq, wk, wv, wp)
    res = run_bass_kernel_spmd(nc, maps, list(range(NCORES))).results
    out = np.concatenate([res[c]["out"] for c in range(NCORES)], axis=0)
    return out.reshape(1, SEQ, DIM).astype(np.float32)
